# revision 1
# baseline (speedup 1.0000x reference)
"""Channelwise symmetric Hausdorff distance loss on 8 Trainium2 NeuronCores.

Math (per (batch, channel) pair; x, y are [N, D] point sets):
    d2[n, m] = |x_n|^2 + |y_m|^2 - 2 x_n.y_m
    h = max( max_n min_m d(n,m), max_m min_n d(n,m) )
    answer   = mean over the B*C pairs of h.

Sharding: B*C = 24 pairs, 3 per NeuronCore (data parallel), host gathers.

Per-core device kernel (per pair):
  - host-prepped inputs: xt = (-2 x)^T fp16 [D, N] (stationary side),
    yt = y^T fp16 [D, N] (moving side), y2a = [y2_hi; y2_lo] fp16 [2, N]
    (hi/lo split of |y_m|^2 so the matmul fold-in keeps fp32-level accuracy),
    x2 = |x_n|^2 fp32 in per-partition layout [128, 8].
  - 16 PSUM blocks [128n, 512m]: 8 accumulating fp16 matmuls (-2 x.y) plus
    one K=2 matmul (ones[2,128]^T @ y2a) that adds y2[m] to every row
    -> psum = y2 - 2 x.y  (fp32).
  - row-min: vector.tensor_reduce min over the free (m) axis straight from
    PSUM -> rowaccs[:, idx].
  - col-min: vector.scalar_tensor_tensor colacc = min(colacc, psum + x2[n])
    (x2 is a per-partition scalar operand) -> colacc[p, m] = min_n d2.
  - outputs per pair: rowaccs [128, 16] fp32, colacc [128, 1024] fp32.
Host finishes in float64: fwd2 = max(x2 + min_mb rowaccs), bwd2 =
max_m(min_p colacc), h = sqrt(max(fwd2, bwd2, 0)), mean over 24 pairs.
"""

import numpy as np

B, C, N, D = 8, 3, 1024, 1024
N_CORES = 8
PAIRS = B * C              # 24
PP = PAIRS // N_CORES      # 3 pairs per core
NT = N // 128              # 8 n-tiles (output partition dim)
MBS = 512                  # m block size (one PSUM bank of fp32)
MB = N // MBS              # 2 m-blocks
KT = D // 128              # 8 k-tiles (contraction)

_NC_CACHE = None


def _legalize_sync(nc):
    """This toolchain's walrus accepts at most ONE sync-wait per instruction;
    Tile emits several (e.g. the tail drain waits on every engine/DMA sem).
    Hoist all but the last wait of each instruction into standalone
    InstEventSemaphore instructions on the same engine, inserted just before
    it — semantically identical (the engine blocks on each in turn)."""
    import concourse.mybir as mybir

    n_split = 0
    for fn in nc.m.functions:
        for bb in fn.blocks:
            new_il = []
            for ins in bb.instructions:
                si = ins.sync_info
                if si is not None and si.on_wait and len(si.on_wait) > 1:
                    waits = list(si.on_wait)
                    for k, w in enumerate(waits[:-1]):
                        ev = mybir.InstEventSemaphore(
                            name=f"{ins.name}-evw{k}",
                            engine=ins.engine,
                            ins=[],
                            outs=[],
                            sync_info=mybir.SyncInfo(on_wait=[w], on_update=[]),
                        )
                        new_il.append(ev)
                        n_split += 1
                    si.on_wait = [waits[-1]]
                new_il.append(ins)
            bb.instructions[:] = new_il
    return n_split


def _build_nc():
    import concourse.bass as bass
    import concourse.mybir as mybir
    import concourse.tile as tile

    f16 = mybir.dt.float16
    f32 = mybir.dt.float32
    f8 = mybir.dt.float8e4
    op_add = mybir.AluOpType.add
    op_min = mybir.AluOpType.min

    nc = bass.Bass("TRN2", target_bir_lowering=True, debug=False)
    xt_d = nc.dram_tensor("xt", [PP, D, N], f8, kind="ExternalInput").ap()
    yt_d = nc.dram_tensor("yt", [PP, D, N], f8, kind="ExternalInput").ap()
    y2a_d = nc.dram_tensor("y2a", [PP, 2, N], f16, kind="ExternalInput").ap()
    x2_d = nc.dram_tensor("x2s", [PP, 128, NT], f32, kind="ExternalInput").ap()
    row_d = nc.dram_tensor(
        "rowout", [PP, 128, NT * MB], f32, kind="ExternalOutput"
    ).ap()
    col_d = nc.dram_tensor("colout", [PP, 128, N], f32, kind="ExternalOutput").ap()

    with tile.TileContext(nc) as tc:
        with (
            tc.tile_pool(name="const", bufs=1) as const_pool,
            tc.tile_pool(name="xy", bufs=2) as xy_pool,
            tc.tile_pool(name="small", bufs=2) as small_pool,
            tc.tile_pool(name="ps", bufs=4, space="PSUM") as ps_pool,
        ):
            ones2 = const_pool.tile([2, 128], f16)
            nc.vector.memset(ones2, 1.0)

            for j in range(PP):
                xt_sb = xy_pool.tile([128, KT, N], f8, tag="xt")
                yt_sb = xy_pool.tile([128, KT, N], f8, tag="yt")
                x2_sb = small_pool.tile([128, NT], f32, tag="x2")
                nc.sync.dma_start(out=x2_sb, in_=x2_d[j])
                y2a_sb = small_pool.tile([2, N], f16, tag="y2a")
                nc.sync.dma_start(out=y2a_sb, in_=y2a_d[j])
                # Per-k-chunk DMAs so the first block's matmuls can start as
                # soon as chunk k has landed instead of after the full 4 MB.
                for k in range(KT):
                    ksl = slice(k * 128, (k + 1) * 128)
                    nc.sync.dma_start(out=xt_sb[:, k, :], in_=xt_d[j, ksl, :])
                    nc.sync.dma_start(out=yt_sb[:, k, :], in_=yt_d[j, ksl, :])

                rowaccs = small_pool.tile([128, NT * MB], f32, tag="rowaccs")
                colacc = small_pool.tile([128, N], f32, tag="colacc")

                for nt in range(NT):
                    nsl = slice(nt * 128, (nt + 1) * 128)
                colacc_v = colacc.rearrange("p (a m) -> p a m", a=MB)
                for nt in range(NT):
                    nsl = slice(nt * 128, (nt + 1) * 128)
                    # Both m-blocks accumulate into one 2-bank PSUM tile so
                    # each stationary operand (xt chunk / ones2) feeds two
                    # back-to-back matmuls (hides LDWEIGHTS) and the DVE can
                    # consume both banks with single fused ops.
                    ps = ps_pool.tile([128, MB, MBS], f32, tag="ps")
                    for ki in range(KT // 2):
                        xsl = xt_sb[:, 2 * ki : 2 * ki + 2, nsl]
                        for mb in range(MB):
                            nc.tensor.matmul(
                                ps[:, mb, :],
                                xsl,
                                yt_sb[:, 2 * ki : 2 * ki + 2, mb * MBS : (mb + 1) * MBS],
                                start=(ki == 0),
                                stop=False,
                                perf_mode=mybir.MatmulPerfMode.DoubleRow,
                            )
                    # += 1*y2_hi[m] + 1*y2_lo[m]  (broadcast over rows)
                    for mb in range(MB):
                        nc.tensor.matmul(
                            ps[:, mb, :],
                            ones2,
                            y2a_sb[:, mb * MBS : (mb + 1) * MBS],
                            start=False,
                            stop=True,
                        )
                    # rowaccs[:, nt*MB:(nt+1)*MB] = min_m (y2[m] - 2 x.y)
                    nc.vector.tensor_reduce(
                        out=rowaccs[:, nt * MB : (nt + 1) * MB],
                        in_=ps,
                        axis=mybir.AxisListType.X,
                        op=op_min,
                    )
                    # colacc = min(colacc, psum + x2[n]) -> min_n d2
                    if nt == 0:
                        nc.vector.tensor_scalar(
                            out=colacc_v,
                            in0=ps,
                            scalar1=x2_sb[:, 0:1],
                            scalar2=None,
                            op0=op_add,
                        )
                    else:
                        nc.vector.scalar_tensor_tensor(
                            out=colacc_v,
                            in0=ps,
                            scalar=x2_sb[:, nt : nt + 1],
                            in1=colacc_v,
                            op0=op_add,
                            op1=op_min,
                        )
                nc.sync.dma_start(out=col_d[j], in_=colacc)
                nc.sync.dma_start(out=row_d[j], in_=rowaccs)
    _legalize_sync(nc)
    return nc


def _prep_inputs(x, y):
    import ml_dtypes

    f8np = np.dtype(ml_dtypes.float8_e4m3)
    x32 = np.ascontiguousarray(x, dtype=np.float32).reshape(PAIRS, N, D)
    y32 = np.ascontiguousarray(y, dtype=np.float32).reshape(PAIRS, N, D)

    xt16 = np.empty((PAIRS, D, N), f8np)
    yt16 = np.empty((PAIRS, D, N), f8np)
    for q in range(PAIRS):
        xt16[q] = (x32[q].T * np.float32(-2.0)).astype(f8np)
        yt16[q] = y32[q].T.astype(f8np)

    x2 = np.square(x32.astype(np.float64)).sum(-1)  # [PAIRS, N]
    y2 = np.square(y32.astype(np.float64)).sum(-1)
    # x2s[q, p, t] = x2[q, t*128 + p]
    x2s = np.ascontiguousarray(
        x2.reshape(PAIRS, NT, 128).transpose(0, 2, 1).astype(np.float32)
    )
    # hi/lo fp16 split of y2: y2 ~ 2048, fp16 hi alone would cost ~1 abs;
    # hi+lo recovers fp32-level accuracy through the matmul fold-in.
    y2_hi = y2.astype(np.float16)
    y2_lo = (y2 - y2_hi.astype(np.float64)).astype(np.float16)
    y2a = np.ascontiguousarray(
        np.stack([y2_hi, y2_lo], axis=1)
    )  # [PAIRS, 2, N] fp16
    return xt16, yt16, x2s, y2a


def _run(x, y, trace=False):
    global _NC_CACHE
    from concourse.bass_utils import run_bass_kernel_spmd

    xt16, yt16, x2s, y2a = _prep_inputs(x, y)

    if _NC_CACHE is None:
        _NC_CACHE = _build_nc()
    nc = _NC_CACHE

    in_maps = []
    for i in range(N_CORES):
        q0 = i * PP
        in_maps.append(
            {
                "xt": xt16[q0 : q0 + PP],
                "yt": yt16[q0 : q0 + PP],
                "y2a": y2a[q0 : q0 + PP],
                "x2s": x2s[q0 : q0 + PP],
            }
        )

    res = run_bass_kernel_spmd(nc, in_maps, core_ids=list(range(N_CORES)), trace=trace)

    h2 = np.empty(PAIRS, np.float64)
    for i in range(N_CORES):
        r = res.results[i]
        for j in range(PP):
            q = i * PP + j
            # rowaccs: [128, NT*MB], idx = nt*MB + mb, = min_m(y2 - 2xy)
            rmin = (
                r["rowout"][j].astype(np.float64).reshape(128, NT, MB).min(-1)
            )  # [128, NT]
            fwd2 = (rmin + x2s[q].astype(np.float64)).max()
            # colacc: [128, N] = min over n-tiles of full d2
            bwd2 = r["colout"][j].astype(np.float64).min(0).max()
            h2[q] = max(fwd2, bwd2, 0.0)

    ans = np.sqrt(h2).mean()
    return np.array(ans, dtype=np.float32), res


def kernel(input, target):
    out, _ = _run(np.asarray(input), np.asarray(target), trace=False)
    return out



# revision 13
# speedup vs baseline: 1.0326x; 1.0326x over previous
"""Channelwise symmetric Hausdorff distance loss on 8 Trainium2 NeuronCores.

Math (per (batch, channel) pair; x, y are [N, D] point sets):
    d2[n, m] = |x_n|^2 + |y_m|^2 - 2 x_n.y_m
    h = max( max_n min_m d(n,m), max_m min_n d(n,m) )
    answer   = mean over the B*C pairs of h.

Sharding: B*C = 24 pairs, 3 per NeuronCore (data parallel), host gathers.

v3 design (per pair, per core):
  - host-prepped: xtc/ytc fp8 in DoubleRow chunk layout [4, 128, 2, N]
    (chunk c holds contraction rows [256c, 256c+256), xt pre-scaled by -2),
    y2c fp16 [1, N] = |y_m|^2 - 2048 (centered, single row),
    x2c fp32 [128, NT] = |x_n|^2 - 1024 in per-partition layout.
  - 8 n-blocks, each:
      PE:      8 accumulating fp8-DR matmuls (-2 x.y) + 2 K=1 fp16
               fold-ins (ones^T @ y2c) -> psum = y2c - 2 x.y  (fp32)
      ScalarE: scr16 = cast(psum) to fp16 SBUF (activation Copy)
      DVE:     rowacc[:, b] = min over m of scr16   (tensor_reduce)
               colacc = min(colacc, scr16 + x2c[b]) (fp16 stt, 2x rate)
  - outputs fp16: rowacc [128, NT], colacc [128, N] = min-blocks d2 - 3072.
Host (float64): fwd2 = max(rowacc + 2048 + x2), bwd2 = max_m(min_p colacc
+ 3072), h = sqrt(max(fwd2, bwd2, 0)), mean over 24 pairs.

DMA: xt chunks on sync queue, yt chunks on scalar queue (parallel HWDGE),
small tensors/outputs on gpsimd (SWDGE); per-chunk tiles so the first
matmul only waits on the first 512 KB.
"""

import numpy as np

B, C, N, D = 8, 3, 1024, 1024
N_CORES = 8
PAIRS = B * C              # 24
PP = PAIRS // N_CORES      # 3 pairs per core
NT = N // 128              # 8 n-tiles (output partition dim)
MBS = 512                  # m block size (one PSUM bank of fp32)
MB = N // MBS              # 2 m-blocks
KC = 4                     # DoubleRow k-chunks (each 256 contraction rows)

Y2OFF = 2048.0             # host-side centering constants
X2OFF = 1024.0

_NC_CACHE = None


def _legalize_sync(nc):
    """This toolchain's walrus accepts at most ONE sync-wait per instruction;
    Tile emits several (e.g. the tail drain waits on every engine/DMA sem).
    Hoist all but the last wait of each instruction into standalone
    InstEventSemaphore instructions on the same engine, inserted just before
    it — semantically identical (the engine blocks on each in turn)."""
    import concourse.mybir as mybir

    n_split = 0
    for fn in nc.m.functions:
        for bb in fn.blocks:
            new_il = []
            for ins in bb.instructions:
                si = ins.sync_info
                if si is not None and si.on_wait and len(si.on_wait) > 1:
                    waits = list(si.on_wait)
                    for k, w in enumerate(waits[:-1]):
                        ev = mybir.InstEventSemaphore(
                            name=f"{ins.name}-evw{k}",
                            engine=ins.engine,
                            ins=[],
                            outs=[],
                            sync_info=mybir.SyncInfo(on_wait=[w], on_update=[]),
                        )
                        new_il.append(ev)
                        n_split += 1
                    si.on_wait = [waits[-1]]
                new_il.append(ins)
            bb.instructions[:] = new_il
    return n_split


def _build_nc():
    import concourse.bass as bass
    import concourse.mybir as mybir
    import concourse.tile as tile

    f16 = mybir.dt.float16
    f32 = mybir.dt.float32
    f8 = mybir.dt.float8e4
    op_add = mybir.AluOpType.add
    op_min = mybir.AluOpType.min

    nc = bass.Bass("TRN2", target_bir_lowering=True, debug=False)
    xtc_d = nc.dram_tensor("xtc", [PP, KC, 128, 2, N], f8, kind="ExternalInput").ap()
    ytc_d = nc.dram_tensor("ytc", [PP, KC, 128, 2, N], f8, kind="ExternalInput").ap()
    y2c_d = nc.dram_tensor("y2c", [PP, 1, N], f16, kind="ExternalInput").ap()
    x2_d = nc.dram_tensor("x2c", [PP, 128, NT], f32, kind="ExternalInput").ap()
    row_d = nc.dram_tensor("rowout", [PP, 128, NT], f16, kind="ExternalOutput").ap()
    col_d = nc.dram_tensor("colout", [PP, 128, N], f16, kind="ExternalOutput").ap()

    with tile.TileContext(nc) as tc:
        with (
            tc.tile_pool(name="const", bufs=1) as const_pool,
            tc.tile_pool(name="xy", bufs=2) as xy_pool,
            tc.tile_pool(name="small", bufs=2) as small_pool,
            tc.tile_pool(name="scr", bufs=4) as scr_pool,
            tc.tile_pool(name="ps", bufs=4, space="PSUM") as ps_pool,
        ):
            ones1 = const_pool.tile([1, 128], f16)
            nc.vector.memset(ones1, 1.0)

            for j in range(PP):
                y2c_sb = small_pool.tile([1, N], f16, tag="y2c")
                nc.gpsimd.dma_start(out=y2c_sb, in_=y2c_d[j])
                x2_sb = small_pool.tile([128, NT], f32, tag="x2")
                nc.gpsimd.dma_start(out=x2_sb, in_=x2_d[j])

                xc, yc = [], []
                for c in range(KC):
                    xt_c = xy_pool.tile([128, 2, N], f8, tag=f"xt{c}")
                    yt_c = xy_pool.tile([128, 2, N], f8, tag=f"yt{c}")
                    nc.sync.dma_start(out=xt_c, in_=xtc_d[j, c])
                    nc.scalar.dma_start(out=yt_c, in_=ytc_d[j, c])
                    xc.append(xt_c)
                    yc.append(yt_c)

                rowacc = small_pool.tile([128, NT], f16, tag="rowacc")
                colacc = small_pool.tile([128, N], f16, tag="colacc")

                for nt in range(NT):
                    nsl = slice(nt * 128, (nt + 1) * 128)
                    # [128, N] fp32 = 2 PSUM banks, address-contiguous
                    ps = ps_pool.tile([128, N], f32, tag="ps")
                    for ki in range(KC):
                        xsl = xc[ki][:, :, nsl]
                        for mb in range(MB):
                            nc.tensor.matmul(
                                ps[:, mb * MBS : (mb + 1) * MBS],
                                xsl,
                                yc[ki][:, :, mb * MBS : (mb + 1) * MBS],
                                start=(ki == 0),
                                stop=False,
                                perf_mode=mybir.MatmulPerfMode.DoubleRow,
                            )
                    # += 1 * y2c[m] (broadcast over rows): psum = y2c - 2 x.y
                    for mb in range(MB):
                        nc.tensor.matmul(
                            ps[:, mb * MBS : (mb + 1) * MBS],
                            ones1,
                            y2c_sb[:, mb * MBS : (mb + 1) * MBS],
                            start=False,
                            stop=True,
                        )
                    # ScalarE: scr16 = cast(psum) -> fp16 SBUF
                    scr = scr_pool.tile([128, N], f16, tag="scr")
                    nc.scalar.copy(out=scr, in_=ps)
                    # rowacc[:, nt] = min over m of scr16
                    nc.vector.tensor_reduce(
                        out=rowacc[:, nt : nt + 1],
                        in_=scr,
                        axis=mybir.AxisListType.X,
                        op=op_min,
                    )
                    # colacc = min(colacc, scr16 + x2c[nt])  (all fp16, 2x)
                    if nt == 0:
                        nc.vector.tensor_scalar(
                            out=colacc,
                            in0=scr,
                            scalar1=x2_sb[:, 0:1],
                            scalar2=None,
                            op0=op_add,
                        )
                    else:
                        nc.vector.scalar_tensor_tensor(
                            out=colacc,
                            in0=scr,
                            scalar=x2_sb[:, nt : nt + 1],
                            in1=colacc,
                            op0=op_add,
                            op1=op_min,
                        )
                nc.gpsimd.dma_start(out=row_d[j], in_=rowacc)
                nc.gpsimd.dma_start(out=col_d[j], in_=colacc)
    _legalize_sync(nc)
    return nc


def _prep_inputs(x, y):
    import ml_dtypes

    f8np = np.dtype(ml_dtypes.float8_e4m3)
    x32 = np.ascontiguousarray(x, dtype=np.float32).reshape(PAIRS, N, D)
    y32 = np.ascontiguousarray(y, dtype=np.float32).reshape(PAIRS, N, D)

    # fp8 chunk layout [PAIRS, KC, 128, 2, N]: element [q, c, p, o, n] =
    # op[q][k = 256c + 128o + p, n] where xt = (-2 x)^T, yt = y^T.
    xt8 = (x32.transpose(0, 2, 1) * np.float32(-2.0)).astype(f8np)  # [q, D, N]
    yt8 = y32.transpose(0, 2, 1).astype(f8np)
    xtc = np.ascontiguousarray(xt8.reshape(PAIRS, KC, 2, 128, N).transpose(0, 1, 3, 2, 4))
    ytc = np.ascontiguousarray(yt8.reshape(PAIRS, KC, 2, 128, N).transpose(0, 1, 3, 2, 4))

    x2 = np.square(x32.astype(np.float64)).sum(-1)  # [PAIRS, N]
    y2 = np.square(y32.astype(np.float64)).sum(-1)
    # x2c[q, p, t] = x2[q, t*128 + p] - X2OFF   (fp16, centered)
    x2c = np.ascontiguousarray(
        (x2 - X2OFF).reshape(PAIRS, NT, 128).transpose(0, 2, 1).astype(np.float32)
    )
    # y2c[q, 0, m] = y2[q, m] - Y2OFF  (single row, fp16)
    y2c = np.ascontiguousarray((y2 - Y2OFF).astype(np.float16)[:, None, :])
    return xtc, ytc, x2c, y2c, x2


def _run(x, y, trace=False):
    global _NC_CACHE
    from concourse.bass_utils import run_bass_kernel_spmd

    xtc, ytc, x2c, y2c, x2 = _prep_inputs(x, y)

    if _NC_CACHE is None:
        _NC_CACHE = _build_nc()
    nc = _NC_CACHE

    in_maps = []
    for i in range(N_CORES):
        q0 = i * PP
        in_maps.append(
            {
                "xtc": xtc[q0 : q0 + PP],
                "ytc": ytc[q0 : q0 + PP],
                "y2c": y2c[q0 : q0 + PP],
                "x2c": x2c[q0 : q0 + PP],
            }
        )

    res = run_bass_kernel_spmd(nc, in_maps, core_ids=list(range(N_CORES)), trace=trace)

    h2 = np.empty(PAIRS, np.float64)
    for i in range(N_CORES):
        r = res.results[i]
        for j in range(PP):
            q = i * PP + j
            # rowacc[p, t] = min_m(y2 - 2xy) - Y2OFF for n = t*128 + p
            row = r["rowout"][j].astype(np.float64)  # [128, NT]
            x2p = x2[q].reshape(NT, 128).T           # [128, NT] exact
            fwd2 = (row + Y2OFF + x2p).max()
            # colacc[p, m] = min over blocks of d2 - (X2OFF + Y2OFF)
            col = r["colout"][j].astype(np.float64)  # [128, N]
            bwd2 = (col.min(0) + (X2OFF + Y2OFF)).max()
            h2[q] = max(fwd2, bwd2, 0.0)

    ans = np.sqrt(h2).mean()
    return np.array(ans, dtype=np.float32), res


def kernel(input, target):
    out, _ = _run(np.asarray(input), np.asarray(target), trace=False)
    return out


# revision 19
# speedup vs baseline: 1.0817x; 1.0475x over previous
"""Channelwise symmetric Hausdorff distance loss on 8 Trainium2 NeuronCores.

Math (per (batch, channel) pair; x, y are [N, D] point sets):
    d2[n, m] = |x_n|^2 + |y_m|^2 - 2 x_n.y_m
    h = max( max_n min_m d(n,m), max_m min_n d(n,m) )
    answer   = mean over the B*C pairs of h.

Sharding: B*C = 24 pairs, 3 per NeuronCore (data parallel), host gathers.

v3 design (per pair, per core):
  - host-prepped: xtc/ytc fp8 in DoubleRow chunk layout [4, 128, 2, N]
    (chunk c holds contraction rows [256c, 256c+256), xt pre-scaled by -2),
    y2c fp16 [1, N] = |y_m|^2 - 2048 (centered, single row),
    x2c fp32 [128, NT] = |x_n|^2 - 1024 in per-partition layout.
  - ~28 K=1 warmup matmuls run during the initial DMA fill so the PE HAM
    clock-gate is already at 2.4 GHz when real matmuls start.
  - 8 n-blocks, each:
      PE:      8 accumulating fp8-DR matmuls (-2 x.y) + 2 K=1 fp16
               fold-ins (ones^T @ y2c) packed CONCURRENTLY on array rows
               0 and 32 -> psum = y2c - 2 x.y  (fp32)
      ScalarE: scr2 = cast(psum + x2c[b]) to fp16 (activation Identity
               with per-partition bias) = d2 - 3072
      DVE:     rowacc[:, b] = min over m of scr2   (tensor_reduce)
      DVE/GpSimd (alternating blocks): colacc_{e,o} = min(colacc, scr2)
  - outputs fp16: rowacc [128, NT], colacc [2, 128, N]; all = d2 - 3072.
Host (float64): fwd2 = max(rowacc) + 3072, bwd2 = max_m(min_p min(colacc_e,
colacc_o)) + 3072, h = sqrt(max(fwd2, bwd2, 0)), mean over 24 pairs.

DMA: xt chunks on sync queue, yt chunks on scalar queue (parallel HWDGE),
small tensors/outputs on gpsimd (SWDGE); per-chunk tiles so the first
matmul only waits on the first 512 KB.
"""

import numpy as np

B, C, N, D = 8, 3, 1024, 1024
N_CORES = 8
PAIRS = B * C              # 24
PP = PAIRS // N_CORES      # 3 pairs per core
NT = N // 128              # 8 n-tiles (output partition dim)
MBS = 512                  # m block size (one PSUM bank of fp32)
MB = N // MBS              # 2 m-blocks
KC = 4                     # DoubleRow k-chunks (each 256 contraction rows)

Y2OFF = 2048.0             # host-side centering constants
X2OFF = 1024.0

_NC_CACHE = None


def _legalize_sync(nc):
    """This toolchain's walrus accepts at most ONE sync-wait per instruction;
    Tile emits several (e.g. the tail drain waits on every engine/DMA sem).
    Hoist all but the last wait of each instruction into standalone
    InstEventSemaphore instructions on the same engine, inserted just before
    it — semantically identical (the engine blocks on each in turn)."""
    import concourse.mybir as mybir

    n_split = 0
    for fn in nc.m.functions:
        for bb in fn.blocks:
            new_il = []
            for ins in bb.instructions:
                si = ins.sync_info
                if si is not None and si.on_wait and len(si.on_wait) > 1:
                    waits = list(si.on_wait)
                    for k, w in enumerate(waits[:-1]):
                        ev = mybir.InstEventSemaphore(
                            name=f"{ins.name}-evw{k}",
                            engine=ins.engine,
                            ins=[],
                            outs=[],
                            sync_info=mybir.SyncInfo(on_wait=[w], on_update=[]),
                        )
                        new_il.append(ev)
                        n_split += 1
                    si.on_wait = [waits[-1]]
                new_il.append(ins)
            bb.instructions[:] = new_il
    return n_split


def _build_nc():
    import concourse.bass as bass
    import concourse.mybir as mybir
    import concourse.tile as tile

    f16 = mybir.dt.float16
    f32 = mybir.dt.float32
    f8 = mybir.dt.float8e4
    op_add = mybir.AluOpType.add
    op_min = mybir.AluOpType.min

    nc = bass.Bass("TRN2", target_bir_lowering=True, debug=False)
    xtc_d = nc.dram_tensor("xtc", [PP, KC, 128, 2, N], f8, kind="ExternalInput").ap()
    ytc_d = nc.dram_tensor("ytc", [PP, KC, 128, 2, N], f8, kind="ExternalInput").ap()
    y2c_d = nc.dram_tensor("y2c", [PP, 1, N], f16, kind="ExternalInput").ap()
    x2_d = nc.dram_tensor("x2c", [PP, 128, NT], f32, kind="ExternalInput").ap()
    row_d = nc.dram_tensor("rowout", [PP, 128, NT], f16, kind="ExternalOutput").ap()
    col_d = nc.dram_tensor("colout", [PP, 2, 128, N], f16, kind="ExternalOutput").ap()

    with tile.TileContext(nc) as tc:
        with (
            tc.tile_pool(name="const", bufs=1) as const_pool,
            tc.tile_pool(name="xy", bufs=2) as xy_pool,
            tc.tile_pool(name="small", bufs=2) as small_pool,
            tc.tile_pool(name="scr", bufs=4) as scr_pool,
            tc.tile_pool(name="ps", bufs=3, space="PSUM") as ps_pool,
            tc.tile_pool(name="warm", bufs=1, space="PSUM") as warm_pool,
        ):
            # ones on partitions 0 and 32 (concurrent fold-in row tiles)
            ones_t = const_pool.tile([33, 128], f16)
            nc.vector.memset(ones_t, 1.0)

            # HAM warm-up: keep the PE busy during the initial DMA fill so
            # the clock gate reaches 8/8 before the first real matmul.
            wps = warm_pool.tile([128, 128], f32)
            for w in range(28):
                nc.tensor.matmul(
                    wps, ones_t[0:1, :], ones_t[0:1, :], start=True, stop=True
                )

            for j in range(PP):
                y2c_sb = small_pool.tile([33, N], f16, tag="y2c")
                nc.gpsimd.dma_start(out=y2c_sb[0:1, :], in_=y2c_d[j])
                nc.gpsimd.dma_start(out=y2c_sb[32:33, :], in_=y2c_d[j])
                x2_sb = small_pool.tile([128, NT], f32, tag="x2")
                nc.gpsimd.dma_start(out=x2_sb, in_=x2_d[j])

                xc, yc = [], []
                for c in range(KC):
                    xt_c = xy_pool.tile([128, 2, N], f8, tag=f"xt{c}")
                    yt_c = xy_pool.tile([128, 2, N], f8, tag=f"yt{c}")
                    nc.sync.dma_start(out=xt_c, in_=xtc_d[j, c])
                    nc.scalar.dma_start(out=yt_c, in_=ytc_d[j, c])
                    xc.append(xt_c)
                    yc.append(yt_c)

                rowacc = small_pool.tile([128, NT], f16, tag="rowacc")
                colacc_e = small_pool.tile([128, N], f16, tag="colacc_e")
                colacc_o = small_pool.tile([128, N], f16, tag="colacc_o")

                for nt in range(NT):
                    nsl = slice(nt * 128, (nt + 1) * 128)
                    # [128, N] fp32 = 2 PSUM banks, address-contiguous
                    ps = ps_pool.tile([128, N], f32, tag="ps")
                    for ki in range(KC):
                        xsl = xc[ki][:, :, nsl]
                        for mb in range(MB):
                            nc.tensor.matmul(
                                ps[:, mb * MBS : (mb + 1) * MBS],
                                xsl,
                                yc[ki][:, :, mb * MBS : (mb + 1) * MBS],
                                start=(ki == 0),
                                stop=False,
                                perf_mode=mybir.MatmulPerfMode.DoubleRow,
                            )
                    # += 1 * y2c[m] (broadcast over rows): psum = y2c - 2 x.y
                    # Two K=1 fold-ins packed on array rows 0 and 32 so they
                    # run concurrently (~1x fold-in cost instead of 2x).
                    nc.tensor.matmul(
                        ps[:, 0:MBS],
                        ones_t[0:1, :],
                        y2c_sb[0:1, 0:MBS],
                        start=False,
                        stop=True,
                    )
                    nc.tensor.matmul(
                        ps[:, MBS : 2 * MBS],
                        ones_t[32:33, :],
                        y2c_sb[32:33, MBS : 2 * MBS],
                        start=False,
                        stop=True,
                    )
                    # ScalarE: scr2 = fp16(psum + x2c[nt]) = d2 - 3072
                    scr = scr_pool.tile([128, N], f16, tag="scr")
                    nc.scalar.activation(
                        out=scr,
                        in_=ps,
                        func=mybir.ActivationFunctionType.Identity,
                        bias=x2_sb[:, nt : nt + 1],
                        scale=1.0,
                    )
                    # rowacc[:, nt] = min over m of scr2
                    nc.vector.tensor_reduce(
                        out=rowacc[:, nt : nt + 1],
                        in_=scr,
                        axis=mybir.AxisListType.X,
                        op=op_min,
                    )
                    # col path: two accumulators on VE, merged on host
                    if nt == 0:
                        nc.vector.tensor_copy(colacc_e, scr)
                    elif nt == 1:
                        nc.vector.tensor_copy(colacc_o, scr)
                    elif nt % 2 == 0:
                        nc.vector.tensor_tensor(colacc_e, colacc_e, scr, op_min)
                    else:
                        nc.vector.tensor_tensor(colacc_o, colacc_o, scr, op_min)
                nc.gpsimd.dma_start(out=row_d[j], in_=rowacc)
                nc.gpsimd.dma_start(out=col_d[j, 0], in_=colacc_e)
                nc.gpsimd.dma_start(out=col_d[j, 1], in_=colacc_o)
    _legalize_sync(nc)
    return nc


def _prep_inputs(x, y):
    import ml_dtypes

    f8np = np.dtype(ml_dtypes.float8_e4m3)
    x32 = np.ascontiguousarray(x, dtype=np.float32).reshape(PAIRS, N, D)
    y32 = np.ascontiguousarray(y, dtype=np.float32).reshape(PAIRS, N, D)

    # fp8 chunk layout [PAIRS, KC, 128, 2, N]: element [q, c, p, o, n] =
    # op[q][k = 256c + 128o + p, n] where xt = (-2 x)^T, yt = y^T.
    xt8 = (x32.transpose(0, 2, 1) * np.float32(-2.0)).astype(f8np)  # [q, D, N]
    yt8 = y32.transpose(0, 2, 1).astype(f8np)
    xtc = np.ascontiguousarray(xt8.reshape(PAIRS, KC, 2, 128, N).transpose(0, 1, 3, 2, 4))
    ytc = np.ascontiguousarray(yt8.reshape(PAIRS, KC, 2, 128, N).transpose(0, 1, 3, 2, 4))

    x2 = np.square(x32.astype(np.float64)).sum(-1)  # [PAIRS, N]
    y2 = np.square(y32.astype(np.float64)).sum(-1)
    # x2c[q, p, t] = x2[q, t*128 + p] - X2OFF   (fp16, centered)
    x2c = np.ascontiguousarray(
        (x2 - X2OFF).reshape(PAIRS, NT, 128).transpose(0, 2, 1).astype(np.float32)
    )
    # y2c[q, 0, m] = y2[q, m] - Y2OFF  (single row, fp16)
    y2c = np.ascontiguousarray((y2 - Y2OFF).astype(np.float16)[:, None, :])
    return xtc, ytc, x2c, y2c, x2


def _run(x, y, trace=False):
    global _NC_CACHE
    from concourse.bass_utils import run_bass_kernel_spmd

    xtc, ytc, x2c, y2c, x2 = _prep_inputs(x, y)

    if _NC_CACHE is None:
        _NC_CACHE = _build_nc()
    nc = _NC_CACHE

    in_maps = []
    for i in range(N_CORES):
        q0 = i * PP
        in_maps.append(
            {
                "xtc": xtc[q0 : q0 + PP],
                "ytc": ytc[q0 : q0 + PP],
                "y2c": y2c[q0 : q0 + PP],
                "x2c": x2c[q0 : q0 + PP],
            }
        )

    res = run_bass_kernel_spmd(nc, in_maps, core_ids=list(range(N_CORES)), trace=trace)

    h2 = np.empty(PAIRS, np.float64)
    for i in range(N_CORES):
        r = res.results[i]
        for j in range(PP):
            q = i * PP + j
            # rowacc[p, t] = min_m d2 - 3072 for n = t*128 + p
            row = r["rowout"][j].astype(np.float64)  # [128, NT]
            fwd2 = row.max() + (X2OFF + Y2OFF)
            # colacc[v, p, m] = min over even/odd blocks of d2 - 3072
            col = r["colout"][j].astype(np.float64)  # [2, 128, N]
            bwd2 = col.min(axis=(0, 1)).max() + (X2OFF + Y2OFF)
            h2[q] = max(fwd2, bwd2, 0.0)

    ans = np.sqrt(h2).mean()
    return np.array(ans, dtype=np.float32), res


def kernel(input, target):
    out, _ = _run(np.asarray(input), np.asarray(target), trace=False)
    return out


# revision 22
# speedup vs baseline: 1.1018x; 1.0186x over previous
"""Channelwise symmetric Hausdorff distance loss on 8 Trainium2 NeuronCores.

Math (per (batch, channel) pair; x, y are [N, D] point sets):
    d2[n, m] = |x_n|^2 + |y_m|^2 - 2 x_n.y_m
    h = max( max_n min_m d(n,m), max_m min_n d(n,m) )
    answer   = mean over the B*C pairs of h.

Sharding: B*C = 24 pairs, 3 per NeuronCore (data parallel), host gathers.

v3 design (per pair, per core):
  - host-prepped: xtc/ytc fp8 in DoubleRow chunk layout [4, 128, 2, N]
    (chunk c holds contraction rows [256c, 256c+256), xt pre-scaled by -2),
    y2c fp16 [1, N] = |y_m|^2 - 2048 (centered, single row),
    x2c fp32 [128, NT] = |x_n|^2 - 1024 in per-partition layout.
  - ~28 K=1 warmup matmuls run during the initial DMA fill so the PE HAM
    clock-gate is already at 2.4 GHz when real matmuls start.
  - 8 n-blocks, each:
      PE:      8 accumulating fp8-DR matmuls (-2 x.y) + 2 K=1 fp16
               fold-ins (ones^T @ y2c) packed CONCURRENTLY on array rows
               0 and 32 -> psum = y2c - 2 x.y  (fp32)
      ScalarE: scr2 = cast(psum + x2c[b]) to fp16 (activation Identity
               with per-partition bias) = d2 - 3072
      DVE:     rowacc[:, b] = min over m of scr2   (tensor_reduce)
      DVE/GpSimd (alternating blocks): colacc_{e,o} = min(colacc, scr2)
  - outputs fp16: rowacc [128, NT], colacc [2, 128, N]; all = d2 - 3072.
Host (float64): fwd2 = max(rowacc) + 3072, bwd2 = max_m(min_p min(colacc_e,
colacc_o)) + 3072, h = sqrt(max(fwd2, bwd2, 0)), mean over 24 pairs.

DMA: xt chunks on sync queue, yt chunks on scalar queue (parallel HWDGE),
small tensors/outputs on gpsimd (SWDGE); per-chunk tiles so the first
matmul only waits on the first 512 KB.
"""

import numpy as np

B, C, N, D = 8, 3, 1024, 1024
N_CORES = 8
PAIRS = B * C              # 24
PP = PAIRS // N_CORES      # 3 pairs per core
NT = N // 128              # 8 n-tiles (output partition dim)
MBS = 512                  # m block size (one PSUM bank of fp32)
MB = N // MBS              # 2 m-blocks
KC = 4                     # DoubleRow k-chunks (each 256 contraction rows)

Y2OFF = 2048.0             # host-side centering constants
X2OFF = 1024.0

_NC_CACHE = None


def _legalize_sync(nc):
    """This toolchain's walrus accepts at most ONE sync-wait per instruction;
    Tile emits several (e.g. the tail drain waits on every engine/DMA sem).
    Hoist all but the last wait of each instruction into standalone
    InstEventSemaphore instructions on the same engine, inserted just before
    it — semantically identical (the engine blocks on each in turn)."""
    import concourse.mybir as mybir

    n_split = 0
    for fn in nc.m.functions:
        for bb in fn.blocks:
            new_il = []
            for ins in bb.instructions:
                si = ins.sync_info
                if si is not None and si.on_wait and len(si.on_wait) > 1:
                    waits = list(si.on_wait)
                    for k, w in enumerate(waits[:-1]):
                        ev = mybir.InstEventSemaphore(
                            name=f"{ins.name}-evw{k}",
                            engine=ins.engine,
                            ins=[],
                            outs=[],
                            sync_info=mybir.SyncInfo(on_wait=[w], on_update=[]),
                        )
                        new_il.append(ev)
                        n_split += 1
                    si.on_wait = [waits[-1]]
                new_il.append(ins)
            bb.instructions[:] = new_il
    return n_split


def _build_nc():
    import concourse.bass as bass
    import concourse.mybir as mybir
    import concourse.tile as tile

    f16 = mybir.dt.float16
    f32 = mybir.dt.float32
    f8 = mybir.dt.float8e4
    op_add = mybir.AluOpType.add
    op_min = mybir.AluOpType.min

    nc = bass.Bass("TRN2", target_bir_lowering=True, debug=False)
    xtc_d = nc.dram_tensor("xtc", [PP, KC, 128, 2, N], f8, kind="ExternalInput").ap()
    ytc_d = nc.dram_tensor("ytc", [PP, KC, 128, 2, N], f8, kind="ExternalInput").ap()
    y2c_d = nc.dram_tensor("y2c", [PP, 1, N], f16, kind="ExternalInput").ap()
    x2_d = nc.dram_tensor("x2c", [PP, 128, NT], f32, kind="ExternalInput").ap()
    row_d = nc.dram_tensor("rowout", [PP, 128, NT], f16, kind="ExternalOutput").ap()
    col_d = nc.dram_tensor("colout", [PP, 2, 128, N], f16, kind="ExternalOutput").ap()

    with tile.TileContext(nc) as tc:
        with (
            tc.tile_pool(name="const", bufs=1) as const_pool,
            tc.tile_pool(name="xy", bufs=2) as xy_pool,
            tc.tile_pool(name="small", bufs=2) as small_pool,
            tc.tile_pool(name="scr", bufs=6) as scr_pool,
            tc.tile_pool(name="ps", bufs=4, space="PSUM") as ps_pool,
        ):
            # ones on partitions 0 and 32 (concurrent fold-in row tiles)
            ones_t = const_pool.tile([33, 128], f16)
            nc.vector.memset(ones_t, 1.0)

            # HAM warm-up: keep the PE busy during the initial DMA fill so
            # the clock gate reaches 8/8 before the first real matmul.
            # Uses a regular ps-pool tile slot (released to the rotation).
            wps = ps_pool.tile([128, N], f32, tag="ps")
            for w in range(28):
                nc.tensor.matmul(
                    wps[:, 0:128], ones_t[0:1, :], ones_t[0:1, :],
                    start=True, stop=True,
                )

            for j in range(PP):
                y2c_sb = small_pool.tile([33, N], f16, tag="y2c")
                nc.gpsimd.dma_start(out=y2c_sb[0:1, :], in_=y2c_d[j])
                nc.gpsimd.dma_start(out=y2c_sb[32:33, :], in_=y2c_d[j])
                x2_sb = small_pool.tile([128, NT], f32, tag="x2")
                nc.gpsimd.dma_start(out=x2_sb, in_=x2_d[j])

                xc, yc = [], []
                for c in range(KC):
                    xt_c = xy_pool.tile([128, 2, N], f8, tag=f"xt{c}")
                    yt_c = xy_pool.tile([128, 2, N], f8, tag=f"yt{c}")
                    nc.sync.dma_start(out=xt_c, in_=xtc_d[j, c])
                    nc.sync.dma_start(out=yt_c, in_=ytc_d[j, c])
                    xc.append(xt_c)
                    yc.append(yt_c)

                rowacc = small_pool.tile([128, NT], f16, tag="rowacc")
                colacc_e = small_pool.tile([128, N], f16, tag="colacc_e")
                colacc_o = small_pool.tile([128, N], f16, tag="colacc_o")

                for nt in range(NT):
                    nsl = slice(nt * 128, (nt + 1) * 128)
                    # [128, N] fp32 = 2 PSUM banks, address-contiguous
                    ps = ps_pool.tile([128, N], f32, tag="ps")
                    for ki in range(KC):
                        xsl = xc[ki][:, :, nsl]
                        for mb in range(MB):
                            nc.tensor.matmul(
                                ps[:, mb * MBS : (mb + 1) * MBS],
                                xsl,
                                yc[ki][:, :, mb * MBS : (mb + 1) * MBS],
                                start=(ki == 0),
                                stop=False,
                                perf_mode=mybir.MatmulPerfMode.DoubleRow,
                            )
                    # += 1 * y2c[m] (broadcast over rows): psum = y2c - 2 x.y
                    # Two K=1 fold-ins packed on array rows 0 and 32 so they
                    # run concurrently (~1x fold-in cost instead of 2x).
                    nc.tensor.matmul(
                        ps[:, 0:MBS],
                        ones_t[0:1, :],
                        y2c_sb[0:1, 0:MBS],
                        start=False,
                        stop=True,
                    )
                    nc.tensor.matmul(
                        ps[:, MBS : 2 * MBS],
                        ones_t[32:33, :],
                        y2c_sb[32:33, MBS : 2 * MBS],
                        start=False,
                        stop=True,
                    )
                    # ScalarE: scr2 = fp16(psum + x2c[nt]) = d2 - 3072
                    scr = scr_pool.tile([128, N], f16, tag="scr")
                    nc.scalar.activation(
                        out=scr,
                        in_=ps,
                        func=mybir.ActivationFunctionType.Identity,
                        bias=x2_sb[:, nt : nt + 1],
                        scale=1.0,
                    )
                    # rowacc[:, nt] = min over m of scr2
                    nc.vector.tensor_reduce(
                        out=rowacc[:, nt : nt + 1],
                        in_=scr,
                        axis=mybir.AxisListType.X,
                        op=op_min,
                    )
                    # col path: two accumulators on VE, merged on host
                    if nt == 0:
                        nc.vector.tensor_copy(colacc_e, scr)
                    elif nt == 1:
                        nc.vector.tensor_copy(colacc_o, scr)
                    elif nt % 2 == 0:
                        nc.vector.tensor_tensor(colacc_e, colacc_e, scr, op_min)
                    else:
                        nc.vector.tensor_tensor(colacc_o, colacc_o, scr, op_min)
                    if nt == NT - 2:
                        # colacc_e is final after the last even block
                        nc.sync.dma_start(out=col_d[j, 0], in_=colacc_e)
                nc.sync.dma_start(out=row_d[j], in_=rowacc)
                nc.sync.dma_start(out=col_d[j, 1], in_=colacc_o)
    _legalize_sync(nc)
    return nc


def _prep_inputs(x, y):
    import ml_dtypes

    f8np = np.dtype(ml_dtypes.float8_e4m3)
    x32 = np.ascontiguousarray(x, dtype=np.float32).reshape(PAIRS, N, D)
    y32 = np.ascontiguousarray(y, dtype=np.float32).reshape(PAIRS, N, D)

    # fp8 chunk layout [PAIRS, KC, 128, 2, N]: element [q, c, p, o, n] =
    # op[q][k = 256c + 128o + p, n] where xt = (-2 x)^T, yt = y^T.
    xt8 = (x32.transpose(0, 2, 1) * np.float32(-2.0)).astype(f8np)  # [q, D, N]
    yt8 = y32.transpose(0, 2, 1).astype(f8np)
    xtc = np.ascontiguousarray(xt8.reshape(PAIRS, KC, 2, 128, N).transpose(0, 1, 3, 2, 4))
    ytc = np.ascontiguousarray(yt8.reshape(PAIRS, KC, 2, 128, N).transpose(0, 1, 3, 2, 4))

    x2 = np.square(x32.astype(np.float64)).sum(-1)  # [PAIRS, N]
    y2 = np.square(y32.astype(np.float64)).sum(-1)
    # x2c[q, p, t] = x2[q, t*128 + p] - X2OFF   (fp16, centered)
    x2c = np.ascontiguousarray(
        (x2 - X2OFF).reshape(PAIRS, NT, 128).transpose(0, 2, 1).astype(np.float32)
    )
    # y2c[q, 0, m] = y2[q, m] - Y2OFF  (single row, fp16)
    y2c = np.ascontiguousarray((y2 - Y2OFF).astype(np.float16)[:, None, :])
    return xtc, ytc, x2c, y2c, x2


def _run(x, y, trace=False):
    global _NC_CACHE
    from concourse.bass_utils import run_bass_kernel_spmd

    xtc, ytc, x2c, y2c, x2 = _prep_inputs(x, y)

    if _NC_CACHE is None:
        _NC_CACHE = _build_nc()
    nc = _NC_CACHE

    in_maps = []
    for i in range(N_CORES):
        q0 = i * PP
        in_maps.append(
            {
                "xtc": xtc[q0 : q0 + PP],
                "ytc": ytc[q0 : q0 + PP],
                "y2c": y2c[q0 : q0 + PP],
                "x2c": x2c[q0 : q0 + PP],
            }
        )

    res = run_bass_kernel_spmd(nc, in_maps, core_ids=list(range(N_CORES)), trace=trace)

    h2 = np.empty(PAIRS, np.float64)
    for i in range(N_CORES):
        r = res.results[i]
        for j in range(PP):
            q = i * PP + j
            # rowacc[p, t] = min_m d2 - 3072 for n = t*128 + p
            row = r["rowout"][j].astype(np.float64)  # [128, NT]
            fwd2 = row.max() + (X2OFF + Y2OFF)
            # colacc[v, p, m] = min over even/odd blocks of d2 - 3072
            col = r["colout"][j].astype(np.float64)  # [2, 128, N]
            bwd2 = col.min(axis=(0, 1)).max() + (X2OFF + Y2OFF)
            h2[q] = max(fwd2, bwd2, 0.0)

    ans = np.sqrt(h2).mean()
    return np.array(ans, dtype=np.float32), res


def kernel(input, target):
    out, _ = _run(np.asarray(input), np.asarray(target), trace=False)
    return out
